# revision 8
# baseline (speedup 1.0000x reference)
"""Trainium2 Bass kernel for nn_MinimalAttention (GQA attention block).

Full-input contract: kernel(**inputs) takes the unsharded numpy inputs and
returns the full output. Internally shards across 8 NeuronCores:
  - data-parallel over batch (2) x tensor-parallel over heads (4 groups of
    8 q-heads / 2 kv-heads each), per the TP sharding hint.
  - each core computes a partial [2048, 2048] output (its heads' slice of
    attn_out @ Wo rows); host sums the 4 partials per batch.

Per-core kernel structure (all matmuls bf16, fp32 PSUM accumulation), built
as ONE software-pipelined pass so the PE never idles and ScalarE exp overlaps
matmuls from the start:
  prefix: K-proj (dedup'd; kv-head halves duplicated into kTd via SBUF-SBUF
          DMA), V-proj (vA/vB with ones column for the softmax denominator),
          Q-proj for seq block 0 / head-pair 0.
  16 iterations over (j seq-block, c head-pair): per key chunk kc:
          S^T pair (two 64-contraction row-tiled matmuls) -> ScalarE exp ->
          PV accumulation (lagged 2 chunks), with O-projection chains of the
          previous j-block and the next Q-projection chain interleaved as PE
          filler; normalize via GpSimd partition_broadcast + DVE
          reciprocal_approx_fast.
  tail: O-projection for the last j-block.
"""

import os
import sys

for _p in ("/opt/trn_rl_repo", "/opt/pypackages"):
    if _p not in sys.path and os.path.isdir(_p):
        sys.path.append(_p)

import numpy as np
import ml_dtypes

import concourse.bass as bass
import concourse.bacc as bacc
import concourse.mybir as mybir
import concourse.tile as tile
from concourse.bass_utils import run_bass_kernel_spmd

HIDDEN = 2048
SEQ = 2048
NUM_HEADS = 32
NUM_KV_HEADS = 8
HEAD_DIM = 64
N_CORES = 8
TP = 4                       # head-groups
BATCH = 2
QH = NUM_HEADS // TP         # 8 local q heads -> 4 pairs
KVH = NUM_KV_HEADS // TP     # 2 local kv heads
HC = HIDDEN // 128           # 16 hidden chunks
SC = SEQ // 128              # 16 seq chunks
NJ = SEQ // 512              # 4 seq 512-blocks

BF16 = mybir.dt.bfloat16
F32 = mybir.dt.float32
U16 = mybir.dt.uint16
EXP = mybir.ActivationFunctionType.Exp
SCALE = HEAD_DIM ** -0.5

# exp chunks computed on DVE via the Schraudolph bf16 bit trick instead of
# ScalarE (per-iteration kc indices). Empty = all exp on ScalarE.
DVE_KCS = ()
EXP_A = float(np.float32(128.0 / np.log(2.0) * SCALE))
EXP_B = float(np.float32(1.5 * 2**23 + 16256.0 - 128.0 * 0.0525))

# set by test.py to collect an NTFF profile; harness default = plain run
PROFILE = bool(os.environ.get("KERNEL_PROFILE"))
LAST_EXEC_NS = None
LAST_RESULTS = None


def _body(tc):
    nc = tc.nc
    # host-prepacked layouts: partition dim first, per-partition contiguous
    xt_d = nc.declare_dram_parameter("xtp", [NJ, 128, HC, 512], BF16, isOutput=False)
    wq_d = nc.declare_dram_parameter("wqp", [128, HC, 512], BF16, isOutput=False)
    wk_d = nc.declare_dram_parameter("wkp", [128, HC, 256], BF16, isOutput=False)
    wv_d = nc.declare_dram_parameter("wvp", [128, HC, 128], BF16, isOutput=False)
    wo_d = nc.declare_dram_parameter("wop", [128, 4, HIDDEN], BF16, isOutput=False)
    out = nc.declare_dram_parameter("out", [SEQ, HIDDEN], F32, isOutput=True)

    consts = tc.alloc_tile_pool(name="consts", bufs=1)
    acts = tc.alloc_tile_pool(name="acts", bufs=1)

    # resident weights; issue loads in the order compute needs them
    wk_sb = consts.tile([128, HC, 256], BF16)
    nc.sync.dma_start(out=wk_sb, in_=wk_d[:, :, :])
    wv_sb = consts.tile([128, HC, 128], BF16)
    nc.sync.dma_start(out=wv_sb, in_=wv_d[:, :, :])
    wq_sb = consts.tile([128, HC, 512], BF16)
    nc.sync.dma_start(out=wq_sb, in_=wq_d[:, :, :])
    wo_sb = consts.tile([128, 4, HIDDEN], BF16)
    nc.sync.dma_start(out=wo_sb, in_=wo_d[:, :, :])

    # persistent activations
    qT = [acts.tile([128, SEQ], BF16, name=f"qT{c}") for c in range(4)]
    # kTd[v]: k^T of kv-head v duplicated on both partition halves (rows
    # 0:64 and 64:128) so the row-tiled S pair can read either half.
    kTd = [acts.tile([128, SEQ], BF16, name=f"kTd{v}") for v in range(KVH)]
    # vA: [v | ones | 0*63] -> PV psum rows 0:64 = out^T, row 64 = l
    # vB: [ones | 0*63 | v] -> PV psum row 0 = l, rows 64:128 = out^T
    # (padded to 128 weight columns so LDWEIGHTS gets fast-weight-load)
    vA = [acts.tile([128, SC, 128], BF16, name=f"vA{v}") for v in range(KVH)]
    vB = [acts.tile([128, SC, 128], BF16, name=f"vB{v}") for v in range(KVH)]
    outT = [acts.tile([128, SEQ], BF16, name=f"outT{c}") for c in range(4)]
    for v in range(KVH):
        nc.vector.memset(vA[v][:, :, 64:65], 1.0)
        nc.vector.memset(vA[v][:, :, 65:128], 0.0)
        nc.vector.memset(vB[v][:, :, 0:1], 1.0)
        nc.vector.memset(vB[v][:, :, 1:64], 0.0)
    # normalize staging: rl rows 0/64 hold the raw denominators; e_sb is the
    # 0/1 selector that PE-broadcasts them (psum_r = e_sb.T @ rl); rli holds
    # the broadcast reciprocals.
    e_sb = acts.tile([128, 128], F32, name="e_sb")
    rl = acts.tile([128, 512], F32, name="rl")
    rli = acts.tile([128, 512], F32, name="rli")
    nc.vector.memset(e_sb, 0.0)
    nc.vector.memset(e_sb[64:65, 0:64], 1.0)
    nc.vector.memset(e_sb[0:1, 64:128], 1.0)
    nc.vector.memset(rl, 0.0)

    with tc.tile_pool(name="xt", bufs=2) as xtp:

        def fetch_xt(j):
            t = xtp.tile([128, HC, 512], BF16, tag="xt")
            nc.sync.dma_start(out=t, in_=xt_d[j])
            return t

        # ---- prefix: K (all j), V (all j), Q(0,0) ----
        with tc.tile_pool(name="psA", bufs=2, space="PSUM") as psA:
            xts = [fetch_xt(0), fetch_xt(1)]

            def q_chain_ps(pool, tag, cq, xt):
                ps_q = pool.tile([128, 512], F32, tag=tag)
                for hc in range(HC):
                    nc.tensor.matmul(
                        out=ps_q,
                        lhsT=wq_sb[:, hc, cq * 128:(cq + 1) * 128],
                        rhs=xt[:, hc, :],
                        start=(hc == 0), stop=(hc == HC - 1),
                    )
                return ps_q

            for j in range(NJ):
                xt = xts[j]
                js = slice(j * 512, (j + 1) * 512)
                # K(j): kv-head halves duplicated on host -> [kv0|kv0|kv1|kv1]
                for mk in range(KVH):
                    ps_k = psA.tile([128, 512], F32, tag="pk")
                    for hc in range(HC):
                        nc.tensor.matmul(
                            out=ps_k,
                            lhsT=wk_sb[:, hc, mk * 128:(mk + 1) * 128],
                            rhs=xt[:, hc, :],
                            start=(hc == 0), stop=(hc == HC - 1),
                        )
                    nc.vector.tensor_copy(out=kTd[mk][:, js], in_=ps_k)
                # V(j)
                for m in range(4):
                    ps_v = psA.tile([128, 128], F32, tag="pv")
                    for hc in range(HC):
                        nc.tensor.matmul(
                            out=ps_v,
                            lhsT=xt[:, hc, m * 128:(m + 1) * 128],
                            rhs=wv_sb[:, hc, :],
                            start=(hc == 0), stop=(hc == HC - 1),
                        )
                    kcg = j * 4 + m
                    for v in range(KVH):
                        vs = slice(v * 64, (v + 1) * 64)
                        nc.vector.tensor_copy(out=vA[v][:, kcg, 0:64], in_=ps_v[:, vs])
                        nc.vector.tensor_copy(out=vB[v][:, kcg, 64:128], in_=ps_v[:, vs])
                if j == 0:
                    ps_q = q_chain_ps(psA, "pk", 0, xt)
                    nc.vector.tensor_copy(out=qT[0][:, 0:512], in_=ps_q)
                if j + 2 < NJ:
                    xts.append(fetch_xt(j + 2))
            # xt re-fetch for the first Q filler chains (Q(0,1..3) read x block 0)
            q_xt = {0: fetch_xt(0)}

        # ---- pipelined iterations ----
        with tc.tile_pool(name="put", bufs=2) as put_pool, \
             tc.tile_pool(name="tsp", bufs=2) as ts_pool, \
             tc.tile_pool(name="stage", bufs=2) as stage_pool, \
             tc.tile_pool(name="pss", bufs=2, space="PSUM") as pss, \
             tc.tile_pool(name="pso", bufs=1, space="PSUM") as pso, \
             tc.tile_pool(name="psp", bufs=2, space="PSUM") as psp:

            def oproj_chain(jb, m, n):
                ms = slice((jb * 4 + m) * 128, (jb * 4 + m + 1) * 128)
                ns = slice(n * 512, (n + 1) * 512)
                ps_p = psp.tile([128, 512], F32, tag="pp")
                for cc in range(4):
                    nc.tensor.matmul(
                        out=ps_p,
                        lhsT=outT[cc][:, ms],
                        rhs=wo_sb[:, cc, ns],
                        start=(cc == 0), stop=(cc == 3),
                    )
                st = stage_pool.tile([128, 512], F32, tag="st")
                nc.vector.tensor_copy(out=st, in_=ps_p)
                nc.sync.dma_start(out=out[ms, ns], in_=st)

            for it in range(NJ * 4):
                j, c = divmod(it, 4)
                kv = c // 2
                js = slice(j * 512, (j + 1) * 512)
                # xt prefetch for Q filler chains: iters i..i+3 read block (i+1)//4
                nq = (it + 4 + 1) // 4
                if it % 4 == 0 and nq < NJ and nq not in q_xt:
                    q_xt[nq] = fetch_xt(nq)

                puT = put_pool.tile([128, SC, 2, 512], BF16, tag="puT")
                ps_oA = pso.tile([128, 512], F32, tag="ps_oA")
                ps_oB = pso.tile([128, 512], F32, tag="ps_oB")
                chain_slots = {0: 0, 1: 1, 6: 2, 11: 3} if j > 0 else {}

                def pv_pair(kc):
                    nc.tensor.matmul(
                        out=ps_oA,
                        lhsT=vA[kv][:, kc, :],
                        rhs=puT[:, kc, 0, :],
                        start=(kc == 0), stop=(kc == SC - 1),
                    )
                    nc.tensor.matmul(
                        out=ps_oB,
                        lhsT=vB[kv][:, kc, :],
                        rhs=puT[:, kc, 1, :],
                        start=(kc == 0), stop=(kc == SC - 1),
                    )

                for kc in range(SC):
                    ks = slice(kc * 128, (kc + 1) * 128)
                    ps_s = pss.tile([128, 1024], F32, tag="ps_s")
                    nc.tensor.matmul(
                        out=ps_s[:, 0:512],
                        lhsT=kTd[kv][0:64, ks],
                        rhs=qT[c][0:64, js],
                        start=True, stop=True,
                    )
                    nc.tensor.matmul(
                        out=ps_s[:, 512:1024],
                        lhsT=kTd[kv][64:128, ks],
                        rhs=qT[c][64:128, js],
                        start=True, stop=True,
                    )
                    pu_flat = puT[:, kc].rearrange("p a b -> p (a b)")
                    if kc in DVE_KCS:
                        # Schraudolph: bf16 bits of exp(s*SCALE) appear in the
                        # low u16 of (s*EXP_A + EXP_B) computed in f32
                        tS = ts_pool.tile([128, 1024], F32, tag="tS")
                        nc.vector.tensor_scalar(
                            tS, ps_s, EXP_A, EXP_B,
                            mybir.AluOpType.mult, mybir.AluOpType.add,
                        )
                        lo = tS.bitcast(U16).rearrange("p (n t) -> p n t", t=2)
                        nc.vector.tensor_copy(
                            out=pu_flat.bitcast(U16),
                            in_=lo[:, :, 0:1].rearrange("p n t -> p (n t)"),
                        )
                    else:
                        nc.scalar.activation(out=pu_flat, in_=ps_s, func=EXP, scale=SCALE)
                    if kc >= 2:
                        pv_pair(kc - 2)
                    if kc in chain_slots:
                        oproj_chain(j - 1, c, chain_slots[kc])
                pv_pair(SC - 2)
                pv_pair(SC - 1)

                # normalize: copy the two denominator rows to SBUF, PE-broadcast
                # them to all partitions, reciprocal, then psum*sbuf multiplies
                nc.vector.tensor_copy(out=rl[64:65, :], in_=ps_oA[64:65, :])
                nc.vector.tensor_copy(out=rl[0:1, :], in_=ps_oB[0:1, :])
                ps_r = psp.tile([128, 512], F32, tag="pp")
                nc.tensor.matmul(out=ps_r, lhsT=e_sb, rhs=rl, start=True, stop=True)
                nc.vector.reciprocal(out=rli, in_=ps_r)
                nc.vector.tensor_mul(outT[c][0:64, js], ps_oA[0:64], rli[0:64])
                nc.vector.tensor_mul(outT[c][64:128, js], ps_oB[64:128], rli[64:128])

                # Q filler chain: linear sequence Q(0,1)..Q(3,3), one per iter
                qi = it + 1
                if qi < NJ * 4:
                    jq, cq = divmod(qi, 4)
                    ps_q = q_chain_ps(psp, "pp", cq, q_xt[jq])
                    nc.vector.tensor_copy(
                        out=qT[cq][:, jq * 512:(jq + 1) * 512], in_=ps_q
                    )

            # tail: O-projection of the last j-block
            for m in range(4):
                for n in range(4):
                    oproj_chain(NJ - 1, m, n)

    acts.release()
    consts.release()


_NC_CACHE = None


def _build():
    global _NC_CACHE
    if _NC_CACHE is None:
        nc = bacc.Bacc(
            "TRN2",
            target_bir_lowering=False,
            debug=False,
            enable_asserts=False,
            num_devices=N_CORES,
        )
        with tile.TileContext(nc) as tc:
            _body(tc)
        nc.compile()
        _NC_CACHE = nc
    return _NC_CACHE


def kernel(x, Wq, Wk, Wv, Wo):
    global LAST_EXEC_NS, LAST_RESULTS
    x = np.asarray(x, dtype=np.float32)
    Wq = np.asarray(Wq, dtype=np.float32)
    Wk = np.asarray(Wk, dtype=np.float32)
    Wv = np.asarray(Wv, dtype=np.float32)
    Wo = np.asarray(Wo, dtype=np.float32)
    bf = ml_dtypes.bfloat16

    in_maps = []
    for core in range(N_CORES):
        b, g = divmod(core, TP)
        qs = slice(g * QH * HEAD_DIM, (g + 1) * QH * HEAD_DIM)
        kvs = slice(g * KVH * HEAD_DIM, (g + 1) * KVH * HEAD_DIM)
        # (j, p, o, s): x[b][j*512+s, o*128+p]
        xtp = np.ascontiguousarray(
            x[b].reshape(NJ, 512, HC, 128).transpose(0, 3, 2, 1)
        ).astype(bf)
        wqp = np.ascontiguousarray(
            Wq[:, qs].reshape(HC, 128, 512).transpose(1, 0, 2)
        ).astype(bf)
        wk_g = Wk[:, kvs]
        wkd = np.concatenate(
            [wk_g[:, 0:64], wk_g[:, 0:64], wk_g[:, 64:128], wk_g[:, 64:128]], axis=1
        )
        wkp = np.ascontiguousarray(
            wkd.reshape(HC, 128, 256).transpose(1, 0, 2)
        ).astype(bf)
        wvp = np.ascontiguousarray(
            Wv[:, kvs].reshape(HC, 128, 128).transpose(1, 0, 2)
        ).astype(bf)
        wop = np.ascontiguousarray(
            Wo[qs, :].reshape(4, 128, HIDDEN).transpose(1, 0, 2)
        ).astype(bf)
        in_maps.append({
            "xtp": xtp, "wqp": wqp, "wkp": wkp, "wvp": wvp, "wop": wop,
        })

    nc = _build()
    res = run_bass_kernel_spmd(
        nc,
        in_maps,
        core_ids=list(range(N_CORES)),
        trace=PROFILE,
        trace_cores=list(range(N_CORES)) if PROFILE else None,
    )
    LAST_EXEC_NS = res.exec_time_ns
    LAST_RESULTS = res
    partials = [r["out"] for r in res.results]
    out = np.empty((BATCH, SEQ, HIDDEN), dtype=np.float32)
    for b in range(BATCH):
        out[b] = partials[TP * b]
        for g in range(1, TP):
            out[b] += partials[TP * b + g]
    return out


# revision 12
# speedup vs baseline: 1.1771x; 1.1771x over previous
"""Trainium2 Bass kernel for nn_MinimalAttention (GQA attention block).

Full-input contract: kernel(**inputs) takes the unsharded numpy inputs and
returns the full output. Internally shards across 8 NeuronCores:
  - data-parallel over batch (2) x tensor-parallel over heads (4 groups of
    8 q-heads / 2 kv-heads each), per the TP sharding hint.
  - each core computes a partial [2048, 2048] output (its heads' slice of
    attn_out @ Wo rows); host sums the 4 partials per batch.

Per-core kernel structure (all matmuls bf16, fp32 PSUM accumulation), built
as ONE software-pipelined pass so the PE never idles and ScalarE exp overlaps
matmuls from the start:
  prefix: K-proj (dedup'd; kv-head halves duplicated into kTd via SBUF-SBUF
          DMA), V-proj (vA/vB with ones column for the softmax denominator),
          Q-proj for seq block 0 / head-pair 0.
  16 iterations over (j seq-block, c head-pair): per key chunk kc:
          S^T pair (two 64-contraction row-tiled matmuls) -> ScalarE exp ->
          PV accumulation (lagged 2 chunks), with O-projection chains of the
          previous j-block and the next Q-projection chain interleaved as PE
          filler; normalize via GpSimd partition_broadcast + DVE
          reciprocal_approx_fast.
  tail: O-projection for the last j-block.
"""

import os
import sys

for _p in ("/opt/trn_rl_repo", "/opt/pypackages"):
    if _p not in sys.path and os.path.isdir(_p):
        sys.path.append(_p)

import numpy as np
import ml_dtypes

import concourse.bass as bass
import concourse.bacc as bacc
import concourse.mybir as mybir
import concourse.tile as tile
from concourse.bass_utils import run_bass_kernel_spmd

HIDDEN = 2048
SEQ = 2048
NUM_HEADS = 32
NUM_KV_HEADS = 8
HEAD_DIM = 64
N_CORES = 8
TP = 4                       # head-groups
BATCH = 2
QH = NUM_HEADS // TP         # 8 local q heads -> 4 pairs
KVH = NUM_KV_HEADS // TP     # 2 local kv heads
HC = HIDDEN // 128           # 16 hidden chunks
SC = SEQ // 128              # 16 seq chunks
NJ = SEQ // 512              # 4 seq 512-blocks

BF16 = mybir.dt.bfloat16
F32 = mybir.dt.float32
U16 = mybir.dt.uint16
EXP = mybir.ActivationFunctionType.Exp
SCALE = HEAD_DIM ** -0.5

# exp chunks computed on DVE via the Schraudolph bf16 bit trick instead of
# ScalarE (per-iteration kc indices). Empty = all exp on ScalarE.
DVE_KCS = ()
EXP_A = float(np.float32(128.0 / np.log(2.0) * SCALE))
EXP_B = float(np.float32(1.5 * 2**23 + 16256.0 - 128.0 * 0.0525))

# set by test.py to collect an NTFF profile; harness default = plain run
PROFILE = bool(os.environ.get("KERNEL_PROFILE"))
LAST_EXEC_NS = None
LAST_RESULTS = None


def _body(tc):
    nc = tc.nc
    # host-prepacked layouts: partition dim first, per-partition contiguous
    xt_d = nc.declare_dram_parameter("xtp", [NJ, 128, HC, 512], BF16, isOutput=False)
    wq_d = nc.declare_dram_parameter("wqp", [128, HC, 512], BF16, isOutput=False)
    wk_d = nc.declare_dram_parameter("wkp", [128, HC, 256], BF16, isOutput=False)
    wv_d = nc.declare_dram_parameter("wvp", [128, HC, 128], BF16, isOutput=False)
    wo_d = nc.declare_dram_parameter("wop", [128, 4, HIDDEN], BF16, isOutput=False)
    out = nc.declare_dram_parameter("out", [SEQ, HIDDEN], F32, isOutput=True)

    consts = tc.alloc_tile_pool(name="consts", bufs=1)
    acts = tc.alloc_tile_pool(name="acts", bufs=1)

    # resident weights; issue loads in the order compute needs them
    wk_sb = consts.tile([128, HC, 256], BF16)
    nc.sync.dma_start(out=wk_sb, in_=wk_d[:, :, :])
    wv_sb = consts.tile([128, HC, 128], BF16)
    nc.sync.dma_start(out=wv_sb, in_=wv_d[:, :, :])
    wq_sb = consts.tile([128, HC, 512], BF16)
    nc.sync.dma_start(out=wq_sb, in_=wq_d[:, :, :])
    wo_sb = consts.tile([128, 4, HIDDEN], BF16)
    nc.sync.dma_start(out=wo_sb, in_=wo_d[:, :, :])

    # persistent activations
    qT = [acts.tile([128, SEQ], BF16, name=f"qT{c}") for c in range(4)]
    # kTd[v]: k^T of kv-head v duplicated on both partition halves (rows
    # 0:64 and 64:128) so the row-tiled S pair can read either half.
    kTd = [acts.tile([128, SEQ], BF16, name=f"kTd{v}") for v in range(KVH)]
    # vA: [v | ones | 0*63] -> PV psum rows 0:64 = out^T, row 64 = l
    # vB: [ones | 0*63 | v] -> PV psum row 0 = l, rows 64:128 = out^T
    # (padded to 128 weight columns so LDWEIGHTS gets fast-weight-load)
    vA = [acts.tile([128, SC, 128], BF16, name=f"vA{v}") for v in range(KVH)]
    vB = [acts.tile([128, SC, 128], BF16, name=f"vB{v}") for v in range(KVH)]
    outT = [acts.tile([128, SEQ], BF16, name=f"outT{c}") for c in range(4)]
    for v in range(KVH):
        nc.vector.memset(vA[v][:, :, 64:65], 1.0)
        nc.vector.memset(vA[v][:, :, 65:128], 0.0)
        nc.vector.memset(vB[v][:, :, 0:1], 1.0)
        nc.vector.memset(vB[v][:, :, 1:64], 0.0)
    # normalize staging: rl rows 0/64 hold the raw denominators; e_sb is the
    # 0/1 selector that PE-broadcasts them (psum_r = e_sb.T @ rl); rli holds
    # the broadcast reciprocals. bf16 suffices for the denominators.
    e_sb = acts.tile([128, 128], BF16, name="e_sb")
    rl = acts.tile([128, 512], BF16, name="rl")
    rli = acts.tile([128, 512], F32, name="rli")
    nc.vector.memset(e_sb, 0.0)
    nc.vector.memset(e_sb[64:65, 0:64], 1.0)
    nc.vector.memset(e_sb[0:1, 64:128], 1.0)
    nc.vector.memset(rl, 0.0)

    with tc.tile_pool(name="xt", bufs=2) as xtp:

        def fetch_xt(j):
            t = xtp.tile([128, HC, 512], BF16, tag="xt")
            for q in range(4):
                nc.sync.dma_start(
                    out=t[:, q * 4:(q + 1) * 4, :], in_=xt_d[j, :, q * 4:(q + 1) * 4, :]
                )
            return t

        # ---- prefix: K (all j), V (all j), Q(0,0) ----
        with tc.tile_pool(name="psA", bufs=2, space="PSUM") as psA:
            xts = [fetch_xt(0), fetch_xt(1)]

            def q_chain_ps(pool, tag, cq, xt):
                ps_q = pool.tile([128, 512], F32, tag=tag)
                for hc in range(HC):
                    nc.tensor.matmul(
                        out=ps_q,
                        lhsT=wq_sb[:, hc, cq * 128:(cq + 1) * 128],
                        rhs=xt[:, hc, :],
                        start=(hc == 0), stop=(hc == HC - 1),
                    )
                return ps_q

            for j in range(NJ):
                xt = xts[j]
                js = slice(j * 512, (j + 1) * 512)
                # K(j): kv-head halves duplicated on host -> [kv0|kv0|kv1|kv1]
                for mk in range(KVH):
                    ps_k = psA.tile([128, 512], F32, tag="pk")
                    for hc in range(HC):
                        nc.tensor.matmul(
                            out=ps_k,
                            lhsT=wk_sb[:, hc, mk * 128:(mk + 1) * 128],
                            rhs=xt[:, hc, :],
                            start=(hc == 0), stop=(hc == HC - 1),
                        )
                    nc.vector.tensor_copy(out=kTd[mk][:, js], in_=ps_k)
                # V(j)
                for m in range(4):
                    ps_v = psA.tile([128, 128], F32, tag="pv")
                    for hc in range(HC):
                        nc.tensor.matmul(
                            out=ps_v,
                            lhsT=xt[:, hc, m * 128:(m + 1) * 128],
                            rhs=wv_sb[:, hc, :],
                            start=(hc == 0), stop=(hc == HC - 1),
                        )
                    kcg = j * 4 + m
                    for v in range(KVH):
                        vs = slice(v * 64, (v + 1) * 64)
                        nc.vector.tensor_copy(out=vA[v][:, kcg, 0:64], in_=ps_v[:, vs])
                        nc.vector.tensor_copy(out=vB[v][:, kcg, 64:128], in_=ps_v[:, vs])
                if j == 0:
                    ps_q = q_chain_ps(psA, "pk", 0, xt)
                    nc.vector.tensor_copy(out=qT[0][:, 0:512], in_=ps_q)
                if j + 2 < NJ:
                    xts.append(fetch_xt(j + 2))
            # xt re-fetch for the first Q filler chains (Q(0,1..3) read x block 0)
            q_xt = {0: fetch_xt(0)}

        # ---- flat global-slot software pipeline over 16 (j,c) iterations ----
        # slot n = (it, kc): S-pair(n); PV-pair(n-4); spread filler quanta
        # (O-proj chains of block j-1, next Q-proj chain) so ScalarE exp is
        # never starved and the PV psum pair frees right after its last read.
        with tc.tile_pool(name="put", bufs=2) as put_pool, \
             tc.tile_pool(name="oraw", bufs=1) as oraw_pool, \
             tc.tile_pool(name="stage", bufs=2) as stage_pool, \
             tc.tile_pool(name="pss", bufs=2, space="PSUM") as pss, \
             tc.tile_pool(name="pso", bufs=1, space="PSUM") as pso, \
             tc.tile_pool(name="ppo", bufs=1, space="PSUM") as ppo, \
             tc.tile_pool(name="ppq", bufs=1, space="PSUM") as ppq:

            ITERS = NJ * 4
            puT_t = [None] * ITERS
            pso_t = [None] * ITERS
            oraw_t = [None] * ITERS
            psr_t = [None] * ITERS

            def emit_S(it, kc):
                j, c = divmod(it, 4)
                kv = c // 2
                js = slice(j * 512, (j + 1) * 512)
                if kc == 0:
                    puT_t[it] = put_pool.tile([128, SC, 2, 512], BF16, tag="puT", name="puT")
                ks = slice(kc * 128, (kc + 1) * 128)
                ps_s = pss.tile([128, 1024], F32, tag="ps_s")
                nc.tensor.matmul(
                    out=ps_s[:, 0:512],
                    lhsT=kTd[kv][0:64, ks],
                    rhs=qT[c][0:64, js],
                    start=True, stop=True,
                )
                nc.tensor.matmul(
                    out=ps_s[:, 512:1024],
                    lhsT=kTd[kv][64:128, ks],
                    rhs=qT[c][64:128, js],
                    start=True, stop=True,
                )
                pu_flat = puT_t[it][:, kc].rearrange("p a b -> p (a b)")
                nc.scalar.activation(out=pu_flat, in_=ps_s, func=EXP, scale=SCALE)

            def emit_PV(it, kc):
                j, c = divmod(it, 4)
                kv = c // 2
                if kc == 0:
                    ps_oA = pso.tile([128, 512], F32, tag="oA")
                    ps_oB = pso.tile([128, 512], F32, tag="oB")
                    pso_t[it] = (ps_oA, ps_oB)
                ps_oA, ps_oB = pso_t[it]
                nc.tensor.matmul(
                    out=ps_oA,
                    lhsT=vA[kv][:, kc, :],
                    rhs=puT_t[it][:, kc, 0, :],
                    start=(kc == 0), stop=(kc == SC - 1),
                )
                nc.tensor.matmul(
                    out=ps_oB,
                    lhsT=vB[kv][:, kc, :],
                    rhs=puT_t[it][:, kc, 1, :],
                    start=(kc == 0), stop=(kc == SC - 1),
                )
                if kc == SC - 1:
                    norm_head(it)

            def norm_head(it):
                # free the PV psum pair fast: denominator rows + raw out^T
                # copies; PE-broadcast the denominators into ps_r.
                ps_oA, ps_oB = pso_t[it]
                oa = oraw_pool.tile([128, 512], BF16, tag="ra")
                ob = oraw_pool.tile([128, 512], BF16, tag="rb")
                nc.vector.tensor_copy(out=oa, in_=ps_oA)
                nc.vector.tensor_copy(out=ob, in_=ps_oB)
                oraw_t[it] = (oa, ob)
                nc.vector.tensor_copy(out=rl[64:65, :], in_=oa[64:65, :])
                nc.vector.tensor_copy(out=rl[0:1, :], in_=ob[0:1, :])
                ps_r = ppo.tile([128, 512], F32, tag="ppo")
                nc.tensor.matmul(out=ps_r, lhsT=e_sb, rhs=rl, start=True, stop=True)
                psr_t[it] = ps_r

            def norm_tail(it):
                j, c = divmod(it, 4)
                js = slice(j * 512, (j + 1) * 512)
                oa, ob = oraw_t[it]
                nc.vector.reciprocal(out=rli, in_=psr_t[it])
                nc.vector.tensor_mul(outT[c][0:64, js], oa[0:64], rli[0:64])
                nc.vector.tensor_mul(outT[c][64:128, js], ob[64:128], rli[64:128])

            def oproj_chain(jb, m, n):
                ms = slice((jb * 4 + m) * 128, (jb * 4 + m + 1) * 128)
                ns = slice(n * 512, (n + 1) * 512)
                ps_p = ppo.tile([128, 512], F32, tag="ppo")
                for cc in range(4):
                    nc.tensor.matmul(
                        out=ps_p,
                        lhsT=outT[cc][:, ms],
                        rhs=wo_sb[:, cc, ns],
                        start=(cc == 0), stop=(cc == 3),
                    )
                st = stage_pool.tile([128, 512], F32, tag="st")
                nc.vector.tensor_copy(out=st, in_=ps_p)
                nc.sync.dma_start(out=out[ms, ns], in_=st)

            def q_half(qi, half):
                jq, cq = divmod(qi, 4)
                if half == 0:
                    ps_q = ppq.tile([128, 512], F32, tag="ppq")
                    q_half.ps = ps_q
                else:
                    ps_q = q_half.ps
                for hc in range(half * 8, half * 8 + 8):
                    nc.tensor.matmul(
                        out=ps_q,
                        lhsT=wq_sb[:, hc, cq * 128:(cq + 1) * 128],
                        rhs=xt_for_q[jq][:, hc, :],
                        start=(hc == 0), stop=(hc == HC - 1),
                    )
                if half == 1:
                    nc.vector.tensor_copy(
                        out=qT[cq][:, jq * 512:(jq + 1) * 512], in_=ps_q
                    )

            xt_for_q = q_xt
            for n in range(ITERS * SC):
                it, kc = divmod(n, SC)
                j, c = divmod(it, 4)
                # xt prefetch for upcoming Q chains
                if kc == 0 and j + 1 < NJ and (j + 1) not in q_xt:
                    q_xt[j + 1] = fetch_xt(j + 1)
                emit_S(it, kc)
                if n >= 4:
                    emit_PV(*divmod(n - 4, SC))
                # filler quanta, spread mid-iteration
                qi = it + 1
                # c==0 chains wait for the previous round's last norm_tail
                slots = {10: 1, 12: 2, 14: 3, (15 if c == 0 else 1): 0}
                if j > 0 and kc in slots:
                    oproj_chain(j - 1, c, slots[kc])
                if qi < ITERS and kc == 5:
                    q_half(qi, 0)
                if qi < ITERS and kc == 6:
                    q_half(qi, 1)
                if kc == 4 and it > 0:
                    norm_tail(it - 1)
            # drain: last PV pairs + normalize + final O-projection block
            for nn_ in range(ITERS * SC - 4, ITERS * SC):
                emit_PV(*divmod(nn_, SC))
            norm_tail(ITERS - 1)
            for m in range(4):
                for nn_ in range(4):
                    oproj_chain(NJ - 1, m, nn_)

    acts.release()
    consts.release()


_NC_CACHE = None


def _build():
    global _NC_CACHE
    if _NC_CACHE is None:
        nc = bacc.Bacc(
            "TRN2",
            target_bir_lowering=False,
            debug=False,
            enable_asserts=False,
            num_devices=N_CORES,
        )
        with tile.TileContext(nc) as tc:
            _body(tc)
        nc.compile()
        _NC_CACHE = nc
    return _NC_CACHE


def kernel(x, Wq, Wk, Wv, Wo):
    global LAST_EXEC_NS, LAST_RESULTS
    x = np.asarray(x, dtype=np.float32)
    Wq = np.asarray(Wq, dtype=np.float32)
    Wk = np.asarray(Wk, dtype=np.float32)
    Wv = np.asarray(Wv, dtype=np.float32)
    Wo = np.asarray(Wo, dtype=np.float32)
    bf = ml_dtypes.bfloat16

    in_maps = []
    for core in range(N_CORES):
        b, g = divmod(core, TP)
        qs = slice(g * QH * HEAD_DIM, (g + 1) * QH * HEAD_DIM)
        kvs = slice(g * KVH * HEAD_DIM, (g + 1) * KVH * HEAD_DIM)
        # (j, p, o, s): x[b][j*512+s, o*128+p]
        xtp = np.ascontiguousarray(
            x[b].reshape(NJ, 512, HC, 128).transpose(0, 3, 2, 1)
        ).astype(bf)
        wqp = np.ascontiguousarray(
            Wq[:, qs].reshape(HC, 128, 512).transpose(1, 0, 2)
        ).astype(bf)
        wk_g = Wk[:, kvs]
        wkd = np.concatenate(
            [wk_g[:, 0:64], wk_g[:, 0:64], wk_g[:, 64:128], wk_g[:, 64:128]], axis=1
        )
        wkp = np.ascontiguousarray(
            wkd.reshape(HC, 128, 256).transpose(1, 0, 2)
        ).astype(bf)
        wvp = np.ascontiguousarray(
            Wv[:, kvs].reshape(HC, 128, 128).transpose(1, 0, 2)
        ).astype(bf)
        wop = np.ascontiguousarray(
            Wo[qs, :].reshape(4, 128, HIDDEN).transpose(1, 0, 2)
        ).astype(bf)
        in_maps.append({
            "xtp": xtp, "wqp": wqp, "wkp": wkp, "wvp": wvp, "wop": wop,
        })

    nc = _build()
    res = run_bass_kernel_spmd(
        nc,
        in_maps,
        core_ids=list(range(N_CORES)),
        trace=PROFILE,
        trace_cores=list(range(N_CORES)) if PROFILE else None,
    )
    LAST_EXEC_NS = res.exec_time_ns
    LAST_RESULTS = res
    partials = [r["out"] for r in res.results]
    out = np.empty((BATCH, SEQ, HIDDEN), dtype=np.float32)
    for b in range(BATCH):
        out[b] = partials[TP * b]
        for g in range(1, TP):
            out[b] += partials[TP * b + g]
    return out


# revision 13
# speedup vs baseline: 1.2304x; 1.0452x over previous
"""Trainium2 Bass kernel for nn_MinimalAttention (GQA attention block).

Full-input contract: kernel(**inputs) takes the unsharded numpy inputs and
returns the full output. Internally shards across 8 NeuronCores:
  - data-parallel over batch (2) x tensor-parallel over heads (4 groups of
    8 q-heads / 2 kv-heads each), per the TP sharding hint.
  - each core computes a partial [2048, 2048] output (its heads' slice of
    attn_out @ Wo rows); host sums the 4 partials per batch.

Per-core kernel structure (all matmuls bf16, fp32 PSUM accumulation), built
as ONE software-pipelined pass so the PE never idles and ScalarE exp overlaps
matmuls from the start:
  prefix: K-proj (dedup'd; kv-head halves duplicated into kTd via SBUF-SBUF
          DMA), V-proj (vA/vB with ones column for the softmax denominator),
          Q-proj for seq block 0 / head-pair 0.
  16 iterations over (j seq-block, c head-pair): per key chunk kc:
          S^T pair (two 64-contraction row-tiled matmuls) -> ScalarE exp ->
          PV accumulation (lagged 2 chunks), with O-projection chains of the
          previous j-block and the next Q-projection chain interleaved as PE
          filler; normalize via GpSimd partition_broadcast + DVE
          reciprocal_approx_fast.
  tail: O-projection for the last j-block.
"""

import os
import sys

for _p in ("/opt/trn_rl_repo", "/opt/pypackages"):
    if _p not in sys.path and os.path.isdir(_p):
        sys.path.append(_p)

import numpy as np
import ml_dtypes

import concourse.bass as bass
import concourse.bacc as bacc
import concourse.mybir as mybir
import concourse.tile as tile
from concourse.bass_utils import run_bass_kernel_spmd

HIDDEN = 2048
SEQ = 2048
NUM_HEADS = 32
NUM_KV_HEADS = 8
HEAD_DIM = 64
N_CORES = 8
TP = 4                       # head-groups
BATCH = 2
QH = NUM_HEADS // TP         # 8 local q heads -> 4 pairs
KVH = NUM_KV_HEADS // TP     # 2 local kv heads
HC = HIDDEN // 128           # 16 hidden chunks
SC = SEQ // 128              # 16 seq chunks
NJ = SEQ // 512              # 4 seq 512-blocks

BF16 = mybir.dt.bfloat16
F32 = mybir.dt.float32
U16 = mybir.dt.uint16
EXP = mybir.ActivationFunctionType.Exp
SCALE = HEAD_DIM ** -0.5

# exp chunks computed on DVE via the Schraudolph bf16 bit trick instead of
# ScalarE (per-iteration kc indices). Empty = all exp on ScalarE.
DVE_KCS = ()
EXP_A = float(np.float32(128.0 / np.log(2.0) * SCALE))
EXP_B = float(np.float32(1.5 * 2**23 + 16256.0 - 128.0 * 0.0525))

# set by test.py to collect an NTFF profile; harness default = plain run
PROFILE = bool(os.environ.get("KERNEL_PROFILE"))
LAST_EXEC_NS = None
LAST_RESULTS = None


def _body(tc):
    nc = tc.nc
    # host-prepacked layouts: partition dim first, per-partition contiguous
    xt_d = nc.declare_dram_parameter("xtp", [NJ, 128, HC, 512], BF16, isOutput=False)
    wq_d = nc.declare_dram_parameter("wqp", [128, HC, 512], BF16, isOutput=False)
    wk_d = nc.declare_dram_parameter("wkp", [128, HC, 128], BF16, isOutput=False)
    wv_d = nc.declare_dram_parameter("wvp", [128, HC, 128], BF16, isOutput=False)
    wo_d = nc.declare_dram_parameter("wop", [128, 4, HIDDEN], BF16, isOutput=False)
    out = nc.declare_dram_parameter("out", [SEQ, HIDDEN], F32, isOutput=True)

    consts = tc.alloc_tile_pool(name="consts", bufs=1)
    acts = tc.alloc_tile_pool(name="acts", bufs=1)

    # resident weights; DMAs issued inside the prefix in need order
    wk_sb = consts.tile([128, HC, 128], BF16)
    wv_sb = consts.tile([128, HC, 128], BF16)
    wq_sb = consts.tile([128, HC, 512], BF16)
    wo_sb = consts.tile([128, 4, HIDDEN], BF16)

    # persistent activations
    qT = [acts.tile([128, SEQ], BF16, name=f"qT{c}") for c in range(4)]
    # kTd[v]: k^T of kv-head v duplicated on both partition halves (rows
    # 0:64 and 64:128) so the row-tiled S pair can read either half.
    kTd = [acts.tile([128, SEQ], BF16, name=f"kTd{v}") for v in range(KVH)]
    # vA: [v | ones | 0*63] -> PV psum rows 0:64 = out^T, row 64 = l
    # vB: [ones | 0*63 | v] -> PV psum row 0 = l, rows 64:128 = out^T
    # (padded to 128 weight columns so LDWEIGHTS gets fast-weight-load)
    vA = [acts.tile([128, SC, 128], BF16, name=f"vA{v}") for v in range(KVH)]
    vB = [acts.tile([128, SC, 128], BF16, name=f"vB{v}") for v in range(KVH)]
    outT = [acts.tile([128, SEQ], BF16, name=f"outT{c}") for c in range(4)]
    for v in range(KVH):
        nc.vector.memset(vA[v][:, :, 64:65], 1.0)
        nc.vector.memset(vA[v][:, :, 65:128], 0.0)
        nc.vector.memset(vB[v][:, :, 0:1], 1.0)
        nc.vector.memset(vB[v][:, :, 1:64], 0.0)
    # normalize staging: rl rows 0/64 hold the raw denominators; e_sb is the
    # 0/1 selector that PE-broadcasts them (psum_r = e_sb.T @ rl); rli holds
    # the broadcast reciprocals. bf16 suffices for the denominators.
    e_sb = acts.tile([128, 128], BF16, name="e_sb")
    rl = acts.tile([128, 512], BF16, name="rl")
    rli = acts.tile([128, 512], F32, name="rli")
    nc.vector.memset(e_sb, 0.0)
    nc.vector.memset(e_sb[64:65, 0:64], 1.0)
    nc.vector.memset(e_sb[0:1, 64:128], 1.0)
    nc.vector.memset(rl, 0.0)

    with tc.tile_pool(name="xt", bufs=2) as xtp:

        def fetch_xt(j):
            t = xtp.tile([128, HC, 512], BF16, tag="xt")
            for q in range(4):
                nc.sync.dma_start(
                    out=t[:, q * 4:(q + 1) * 4, :], in_=xt_d[j, :, q * 4:(q + 1) * 4, :]
                )
            return t

        # ---- prefix: K (all j), V (all j), Q(0,0) ----
        with tc.tile_pool(name="psA", bufs=2, space="PSUM") as psA:
            nc.sync.dma_start(out=wk_sb, in_=wk_d[:, :, :])
            xts = [fetch_xt(0)]
            nc.sync.dma_start(out=wv_sb, in_=wv_d[:, :, :])
            xts.append(fetch_xt(1))
            nc.sync.dma_start(out=wq_sb, in_=wq_d[:, :, :])

            def q_chain_ps(pool, tag, cq, xt):
                ps_q = pool.tile([128, 512], F32, tag=tag)
                for hc in range(HC):
                    nc.tensor.matmul(
                        out=ps_q,
                        lhsT=wq_sb[:, hc, cq * 128:(cq + 1) * 128],
                        rhs=xt[:, hc, :],
                        start=(hc == 0), stop=(hc == HC - 1),
                    )
                return ps_q

            for j in range(NJ):
                xt = xts[j]
                js = slice(j * 512, (j + 1) * 512)
                # K(j): single dedup'd 128-col chunk; kv0 dims land on
                # partitions 0:64, kv1 dims on 64:128
                ps_k = psA.tile([128, 512], F32, tag="pk")
                for hc in range(HC):
                    nc.tensor.matmul(
                        out=ps_k,
                        lhsT=wk_sb[:, hc, :],
                        rhs=xt[:, hc, :],
                        start=(hc == 0), stop=(hc == HC - 1),
                    )
                nc.vector.tensor_copy(out=kTd[0][0:64, js], in_=ps_k[0:64, :])
                nc.vector.tensor_copy(out=kTd[1][64:128, js], in_=ps_k[64:128, :])
                # V(j)
                for m in range(4):
                    ps_v = psA.tile([128, 128], F32, tag="pv")
                    for hc in range(HC):
                        nc.tensor.matmul(
                            out=ps_v,
                            lhsT=xt[:, hc, m * 128:(m + 1) * 128],
                            rhs=wv_sb[:, hc, :],
                            start=(hc == 0), stop=(hc == HC - 1),
                        )
                    kcg = j * 4 + m
                    for v in range(KVH):
                        vs = slice(v * 64, (v + 1) * 64)
                        nc.vector.tensor_copy(out=vA[v][:, kcg, 0:64], in_=ps_v[:, vs])
                        nc.vector.tensor_copy(out=vB[v][:, kcg, 64:128], in_=ps_v[:, vs])
                if j == 0:
                    ps_q = q_chain_ps(psA, "pk", 0, xt)
                    nc.vector.tensor_copy(out=qT[0][:, 0:512], in_=ps_q)
                if j + 2 < NJ:
                    xts.append(fetch_xt(j + 2))
            # duplicate each kv head's 64 dims onto the other partition half
            nc.sync.dma_start(out=kTd[0][64:128, :], in_=kTd[0][0:64, :])
            nc.sync.dma_start(out=kTd[1][0:64, :], in_=kTd[1][64:128, :])
            # xt re-fetch for the first Q filler chains (Q(0,1..3) read x block 0)
            q_xt = {0: fetch_xt(0)}
            nc.sync.dma_start(out=wo_sb, in_=wo_d[:, :, :])

        # ---- flat global-slot software pipeline over 16 (j,c) iterations ----
        # slot n = (it, kc): S-pair(n); PV-pair(n-4); spread filler quanta
        # (O-proj chains of block j-1, next Q-proj chain) so ScalarE exp is
        # never starved and the PV psum pair frees right after its last read.
        with tc.tile_pool(name="put", bufs=2) as put_pool, \
             tc.tile_pool(name="oraw", bufs=1) as oraw_pool, \
             tc.tile_pool(name="stage", bufs=2) as stage_pool, \
             tc.tile_pool(name="pss", bufs=2, space="PSUM") as pss, \
             tc.tile_pool(name="pso", bufs=1, space="PSUM") as pso, \
             tc.tile_pool(name="ppo", bufs=1, space="PSUM") as ppo, \
             tc.tile_pool(name="ppq", bufs=1, space="PSUM") as ppq:

            ITERS = NJ * 4
            puT_t = [None] * ITERS
            pso_t = [None] * ITERS
            oraw_t = [None] * ITERS
            psr_t = [None] * ITERS

            def emit_S(it, kc):
                j, c = divmod(it, 4)
                kv = c // 2
                js = slice(j * 512, (j + 1) * 512)
                if kc == 0:
                    puT_t[it] = put_pool.tile([128, SC, 2, 512], BF16, tag="puT", name="puT")
                ks = slice(kc * 128, (kc + 1) * 128)
                ps_s = pss.tile([128, 1024], F32, tag="ps_s")
                nc.tensor.matmul(
                    out=ps_s[:, 0:512],
                    lhsT=kTd[kv][0:64, ks],
                    rhs=qT[c][0:64, js],
                    start=True, stop=True,
                )
                nc.tensor.matmul(
                    out=ps_s[:, 512:1024],
                    lhsT=kTd[kv][64:128, ks],
                    rhs=qT[c][64:128, js],
                    start=True, stop=True,
                )
                pu_flat = puT_t[it][:, kc].rearrange("p a b -> p (a b)")
                nc.scalar.activation(out=pu_flat, in_=ps_s, func=EXP, scale=SCALE)

            def emit_PV(it, kc):
                j, c = divmod(it, 4)
                kv = c // 2
                if kc == 0:
                    ps_oA = pso.tile([128, 512], F32, tag="oA")
                    ps_oB = pso.tile([128, 512], F32, tag="oB")
                    pso_t[it] = (ps_oA, ps_oB)
                ps_oA, ps_oB = pso_t[it]
                nc.tensor.matmul(
                    out=ps_oA,
                    lhsT=vA[kv][:, kc, :],
                    rhs=puT_t[it][:, kc, 0, :],
                    start=(kc == 0), stop=(kc == SC - 1),
                )
                nc.tensor.matmul(
                    out=ps_oB,
                    lhsT=vB[kv][:, kc, :],
                    rhs=puT_t[it][:, kc, 1, :],
                    start=(kc == 0), stop=(kc == SC - 1),
                )
                if kc == SC - 1:
                    norm_head(it)

            def norm_head(it):
                # free the PV psum pair fast: denominator rows + raw out^T
                # copies; PE-broadcast the denominators into ps_r.
                ps_oA, ps_oB = pso_t[it]
                oa = oraw_pool.tile([128, 512], BF16, tag="ra")
                ob = oraw_pool.tile([128, 512], BF16, tag="rb")
                nc.vector.tensor_copy(out=oa, in_=ps_oA)
                nc.vector.tensor_copy(out=ob, in_=ps_oB)
                oraw_t[it] = (oa, ob)
                nc.vector.tensor_copy(out=rl[64:65, :], in_=oa[64:65, :])
                nc.vector.tensor_copy(out=rl[0:1, :], in_=ob[0:1, :])
                ps_r = ppo.tile([128, 512], F32, tag="ppo")
                nc.tensor.matmul(out=ps_r, lhsT=e_sb, rhs=rl, start=True, stop=True)
                psr_t[it] = ps_r

            def norm_tail(it):
                j, c = divmod(it, 4)
                js = slice(j * 512, (j + 1) * 512)
                oa, ob = oraw_t[it]
                nc.vector.reciprocal(out=rli, in_=psr_t[it])
                nc.vector.tensor_mul(outT[c][0:64, js], oa[0:64], rli[0:64])
                nc.vector.tensor_mul(outT[c][64:128, js], ob[64:128], rli[64:128])

            def oproj_chain(jb, m, n, pool=None, tag="ppo"):
                ms = slice((jb * 4 + m) * 128, (jb * 4 + m + 1) * 128)
                ns = slice(n * 512, (n + 1) * 512)
                ps_p = (pool or ppo).tile([128, 512], F32, tag=tag)
                for cc in range(4):
                    nc.tensor.matmul(
                        out=ps_p,
                        lhsT=outT[cc][:, ms],
                        rhs=wo_sb[:, cc, ns],
                        start=(cc == 0), stop=(cc == 3),
                    )
                st = stage_pool.tile([128, 512], F32, tag="st")
                nc.vector.tensor_copy(out=st, in_=ps_p)
                nc.sync.dma_start(out=out[ms, ns], in_=st)

            def q_half(qi, half):
                jq, cq = divmod(qi, 4)
                if half == 0:
                    ps_q = ppq.tile([128, 512], F32, tag="ppq")
                    q_half.ps = ps_q
                else:
                    ps_q = q_half.ps
                for hc in range(half * 8, half * 8 + 8):
                    nc.tensor.matmul(
                        out=ps_q,
                        lhsT=wq_sb[:, hc, cq * 128:(cq + 1) * 128],
                        rhs=xt_for_q[jq][:, hc, :],
                        start=(hc == 0), stop=(hc == HC - 1),
                    )
                if half == 1:
                    nc.vector.tensor_copy(
                        out=qT[cq][:, jq * 512:(jq + 1) * 512], in_=ps_q
                    )

            xt_for_q = q_xt
            for n in range(ITERS * SC):
                it, kc = divmod(n, SC)
                j, c = divmod(it, 4)
                # xt prefetch for upcoming Q chains
                if kc == 0 and j + 1 < NJ and (j + 1) not in q_xt:
                    q_xt[j + 1] = fetch_xt(j + 1)
                emit_S(it, kc)
                if n >= 4:
                    emit_PV(*divmod(n - 4, SC))
                # filler quanta, spread mid-iteration
                qi = it + 1
                # c==0 chains wait for the previous round's last norm_tail
                slots = {10: 1, 12: 2, 14: 3, (15 if c == 0 else 1): 0}
                if j > 0 and kc in slots:
                    oproj_chain(j - 1, c, slots[kc])
                if qi < ITERS and kc == 5:
                    q_half(qi, 0)
                if qi < ITERS and kc == 6:
                    q_half(qi, 1)
                if kc == 4 and it > 0:
                    norm_tail(it - 1)
            # drain: last PV pairs + normalize + final O-projection block
            for nn_ in range(ITERS * SC - 4, ITERS * SC):
                emit_PV(*divmod(nn_, SC))
            norm_tail(ITERS - 1)
            for i, (m, nn_) in enumerate([(m, n) for m in range(4) for n in range(4)]):
                if i % 2 == 0:
                    oproj_chain(NJ - 1, m, nn_)
                else:
                    oproj_chain(NJ - 1, m, nn_, pool=ppq, tag="ppq")

    acts.release()
    consts.release()


_NC_CACHE = None


def _build():
    global _NC_CACHE
    if _NC_CACHE is None:
        nc = bacc.Bacc(
            "TRN2",
            target_bir_lowering=False,
            debug=False,
            enable_asserts=False,
            num_devices=N_CORES,
        )
        with tile.TileContext(nc) as tc:
            _body(tc)
        nc.compile()
        _NC_CACHE = nc
    return _NC_CACHE


def kernel(x, Wq, Wk, Wv, Wo):
    global LAST_EXEC_NS, LAST_RESULTS
    x = np.asarray(x, dtype=np.float32)
    Wq = np.asarray(Wq, dtype=np.float32)
    Wk = np.asarray(Wk, dtype=np.float32)
    Wv = np.asarray(Wv, dtype=np.float32)
    Wo = np.asarray(Wo, dtype=np.float32)
    bf = ml_dtypes.bfloat16

    in_maps = []
    for core in range(N_CORES):
        b, g = divmod(core, TP)
        qs = slice(g * QH * HEAD_DIM, (g + 1) * QH * HEAD_DIM)
        kvs = slice(g * KVH * HEAD_DIM, (g + 1) * KVH * HEAD_DIM)
        # (j, p, o, s): x[b][j*512+s, o*128+p]
        xtp = np.ascontiguousarray(
            x[b].reshape(NJ, 512, HC, 128).transpose(0, 3, 2, 1)
        ).astype(bf)
        wqp = np.ascontiguousarray(
            Wq[:, qs].reshape(HC, 128, 512).transpose(1, 0, 2)
        ).astype(bf)
        wkp = np.ascontiguousarray(
            Wk[:, kvs].reshape(HC, 128, 128).transpose(1, 0, 2)
        ).astype(bf)
        wvp = np.ascontiguousarray(
            Wv[:, kvs].reshape(HC, 128, 128).transpose(1, 0, 2)
        ).astype(bf)
        wop = np.ascontiguousarray(
            Wo[qs, :].reshape(4, 128, HIDDEN).transpose(1, 0, 2)
        ).astype(bf)
        in_maps.append({
            "xtp": xtp, "wqp": wqp, "wkp": wkp, "wvp": wvp, "wop": wop,
        })

    nc = _build()
    res = run_bass_kernel_spmd(
        nc,
        in_maps,
        core_ids=list(range(N_CORES)),
        trace=PROFILE,
        trace_cores=list(range(N_CORES)) if PROFILE else None,
    )
    LAST_EXEC_NS = res.exec_time_ns
    LAST_RESULTS = res
    partials = [r["out"] for r in res.results]
    out = np.empty((BATCH, SEQ, HIDDEN), dtype=np.float32)
    for b in range(BATCH):
        out[b] = partials[TP * b]
        for g in range(1, TP):
            out[b] += partials[TP * b + g]
    return out
